# revision 1
# baseline (speedup 1.0000x reference)
"""nn_CART_69355131895963 Trainium2 Bass kernel.

reference:
    BatchNorm1d(train-mode batch stats) -> per-tree sparsemax feature
    selection (einsum bf,tfs->tbs) -> sigmoid(xp - cut) -> per-tree
    [S,S] MLP layer + relu -> per-tree [S,O] layer -> mean over trees of
    o2 * tw.

Strategy (8 NeuronCores, batch-sharded 8192 rows/core):
  Host (O(params) only): sparsemax(fsm) -> P2 [F,TS]; fold gamma into the
    BN scale, tw/T into W2, build block-diagonal W1 (4 trees/group) and
    stacked W2'; lay every small parameter out for direct SBUF use.
  Device phase 1: stream x fp32 -> cast fp16 (GPSIMD) -> stats matmuls on
    PE (batch-sum via ones-lhsT, sum-of-squares via per-tile self-matmul
    diagonal) -> write x16 to DRAM scratch -> DMA-transpose to xT16 [f,b].
  Device phase 1.5: AllReduce the [2,F] stats across the 8 cores, finish
    mean/var -> rsqrt -> fold into P2a (fp16) and the sigmoid bias.
  Device phase 2 (software-pipelined over 64 (chunk, ts-tile) steps):
    s1: xp_tile = P2a^T @ xT  (PE, fp16, fp32 accum)
    ACT: score = sigmoid(xp + biasA)        (PSUM -> SBUF fp16)
    s2: z2 = W1bd^T @ score                 (PE)
    DVE: o1 = max(z2 + b1, 0)               (PSUM -> SBUF fp16)
    s3: out[16,b] += W2'^T @ o1             (PE, accumulated over 8 groups)
    evict: out + bout (ACT/DVE split) -> DMA to DRAM [16, b]
  Host: concat per-core [16, 8192] outputs along b, transpose -> [B, 16].
"""

import numpy as np

import concourse.tile as tile
from concourse import bacc, mybir
from concourse.bass_utils import run_bass_kernel_spmd

f16 = mybir.dt.float16
f32 = mybir.dt.float32
AF = mybir.ActivationFunctionType
ALU = mybir.AluOpType

N_CORES = 8
B_TOTAL = 65536
BS = B_TOTAL // N_CORES     # 8192 rows per core
F = 256
T = 32
S = 32
O = 16
TS = T * S                  # 1024
NFT = F // 128              # 2 feature tiles
NM = TS // 128              # 8 ts-tiles (tree groups of 4)
BN_EPS = 1e-5
CHUNK = 1024
Q = CHUNK // 512
NCH = BS // CHUNK
ROWS1 = 2048                # phase-1 chunk rows
NCH1 = BS // ROWS1
SUB1 = ROWS1 // 128


def _sparsemax_cols(z):
    """sparsemax along axis 0 of z [F, C] (float64)."""
    zs = np.sort(z, axis=0)[::-1]
    k = np.arange(1, z.shape[0] + 1)[:, None]
    cs = np.cumsum(zs, axis=0)
    support = (1.0 + k * zs) > cs
    ksup = support.sum(0)
    tau = (cs[ksup - 1, np.arange(z.shape[1])] - 1.0) / ksup
    return np.maximum(z - tau, 0.0)


def _host_prep(gamma, beta, fsm, cut, W1, b1, W2, b2, tw):
    P2 = _sparsemax_cols(
        fsm.astype(np.float64).transpose(1, 0, 2).reshape(F, TS)
    ).astype(np.float32)
    p2raw = P2.reshape(NFT, 128, TS).transpose(1, 0, 2).copy()
    cutv = cut.reshape(TS).reshape(NM, 128).T.copy().astype(np.float32)
    b1v = b1.reshape(TS).reshape(NM, 128).T.copy().astype(np.float32)

    w1bd = np.zeros((NM, 128, 128), dtype=np.float32)
    for g in range(NM):
        for i in range(4):
            w1bd[g, 32 * i:32 * i + 32, 32 * i:32 * i + 32] = W1[4 * g + i]
    w1bd = w1bd.transpose(1, 0, 2).astype(np.float16).copy()

    w2f = (W2 * (tw / T)).reshape(TS, O).astype(np.float32) \
        .reshape(NM, 128, O).transpose(1, 0, 2).astype(np.float16).copy()
    bout = (b2 * (tw / T)).sum(0).reshape(O, 1).astype(np.float32)

    gamma2 = gamma.reshape(NFT, 128).T.copy().astype(np.float32)
    beta2 = beta.reshape(NFT, 128).T.copy().astype(np.float32)
    eye = np.eye(128, dtype=np.float32)
    ones16 = np.ones((128, 1), dtype=np.float16)
    return dict(p2raw=p2raw, cutv=cutv, b1v=b1v, w1bd=w1bd, w2f=w2f,
                bout=bout, gamma2=gamma2, beta2=beta2, eye=eye, ones16=ones16)


def build_program(repeat=1, single_core_sim=False):
    """Trace + compile the SPMD Bass program (identical on all 8 cores).

    single_core_sim=True builds the same per-core program with the
    cross-core AllReduce elided (for cost-model simulation only).
    """
    ncores = 1 if single_core_sim else N_CORES
    nc = bacc.Bacc("TRN2", target_bir_lowering=False, debug=False,
                   num_devices=ncores)
    X = nc.dram_tensor("x", [BS, F], f32, kind="ExternalInput").ap()
    P2RAW = nc.dram_tensor("p2raw", [128, NFT, TS], f32, kind="ExternalInput").ap()
    CUTV = nc.dram_tensor("cutv", [128, NM], f32, kind="ExternalInput").ap()
    B1V = nc.dram_tensor("b1v", [128, NM], f32, kind="ExternalInput").ap()
    W1BD = nc.dram_tensor("w1bd", [128, NM, 128], f16, kind="ExternalInput").ap()
    W2F = nc.dram_tensor("w2f", [128, NM, O], f16, kind="ExternalInput").ap()
    BOUT = nc.dram_tensor("bout", [O, 1], f32, kind="ExternalInput").ap()
    GAMMA2 = nc.dram_tensor("gamma2", [128, NFT], f32, kind="ExternalInput").ap()
    BETA2 = nc.dram_tensor("beta2", [128, NFT], f32, kind="ExternalInput").ap()
    EYE = nc.dram_tensor("eye", [128, 128], f32, kind="ExternalInput").ap()
    ONES16 = nc.dram_tensor("ones16", [128, 1], f16, kind="ExternalInput").ap()
    OUT = nc.dram_tensor("out", [O, BS], f32, kind="ExternalOutput").ap()

    Xv = X.rearrange("(n p) f -> p n f", p=128)

    with tile.TileContext(nc) as tc:
        with tc.tile_pool(name="const", bufs=1) as pc, \
             tc.tile_pool(name="xt", bufs=1) as pxt, \
             tc.tile_pool(name="dram", bufs=1, space="DRAM") as pdram:

            def load_const(name, shape, dt, src):
                t = pc.tile(shape, dt, name=name)
                nc.sync.dma_start(t[:], src[:])
                return t

            p2raw = load_const("p2raw_sb", [128, NFT, TS], f32, P2RAW)
            cutv = load_const("cutv_sb", [128, NM], f32, CUTV)
            b1v = load_const("b1v_sb", [128, NM], f32, B1V)
            w1bd = load_const("w1bd_sb", [128, NM, 128], f16, W1BD)
            w2f = load_const("w2f_sb", [128, NM, O], f16, W2F)
            bout = load_const("bout_sb", [O, 1], f32, BOUT)
            gamma2 = load_const("gamma2_sb", [128, NFT], f32, GAMMA2)
            beta2 = load_const("beta2_sb", [128, NFT], f32, BETA2)
            eye = load_const("eye_sb", [128, 128], f32, EYE)
            ones16 = load_const("ones16_sb", [128, 1], f16, ONES16)
            eye16 = pc.tile([128, 128], f16, name="eye16")
            nc.vector.tensor_copy(eye16[:], eye[:])

            xT = [pxt.tile([128, BS], f16, tag=f"xt{i}", name=f"xt{i}")
                  for i in range(NFT)]
            x16d = pdram.tile([NFT, BS, 128], f16)

            def body_once():
                # ---------- phase 1: load, cast fp16, stats, transpose ----
                with tc.tile_pool(name="ph1", bufs=2) as p1, \
                     tc.tile_pool(name="ph1psum", bufs=1, space="PSUM") as pst:
                    sumP = pst.tile([1, F], f32, name="sumP")
                    covP = [pst.tile([128, 128], f32, tag=f"cov{i}",
                                     name=f"cov{i}") for i in range(NFT)]
                    for c in range(NCH1):
                        x32 = p1.tile([128, SUB1, F], f32, tag="x32",
                                      name="x32")
                        nc.sync.dma_start(x32[:],
                                          Xv[:, c * SUB1:(c + 1) * SUB1, :])
                        x16 = p1.tile([128, SUB1, F], f16, tag="x16",
                                      name="x16")
                        nc.gpsimd.tensor_copy(x16[:], x32[:])
                        for i in range(NFT):
                            nc.sync.dma_start(
                                x16d[i, c * ROWS1:(c + 1) * ROWS1, :]
                                  .rearrange("(a p) f -> p a f", p=128),
                                x16[:, :, 128 * i:128 * (i + 1)])
                        for a in range(SUB1):
                            first = (c == 0 and a == 0)
                            last = (c == NCH1 - 1 and a == SUB1 - 1)
                            nc.tensor.matmul(sumP[:], ones16[:],
                                             x16[:, a, :], start=first,
                                             stop=last, skip_group_check=True)
                            for i in range(NFT):
                                sl = x16[:, a, 128 * i:128 * (i + 1)]
                                nc.tensor.matmul(covP[i][:], sl, sl,
                                                 start=first, stop=last,
                                                 skip_group_check=True)
                    stat_sb = pc.tile([128, NFT, 2], f32, name="stat_sb")
                    sum_sb = pc.tile([1, F], f32, name="sum_sb")
                    nc.vector.tensor_copy(sum_sb[:], sumP[:])
                    for i in range(NFT):
                        tmp = p1.tile([128, 128], f32, tag="dtmp", name="dtmp")
                        nc.vector.tensor_tensor(tmp[:], covP[i][:], eye[:],
                                                op=ALU.mult)
                        nc.vector.reduce_sum(stat_sb[:, i, 1:2], tmp[:],
                                             axis=mybir.AxisListType.X)

                # transposes run while the collective is in flight;
                # batched as 2*ROWS1-row calls (fewer, larger transposes
                # cut the per-call serialization cost substantially)
                for c in range(NCH1 // 2):
                    for i in range(NFT):
                        nc.sync.dma_start_transpose(
                            out=xT[i][:, 2 * c * ROWS1:2 * (c + 1) * ROWS1],
                            in_=x16d[i, 2 * c * ROWS1:2 * (c + 1) * ROWS1, :])

                # ---------- phase 1.5: all-reduce + BN fold ----------
                ccin = pdram.tile([2, F], f32, name="ccin")
                ccout = pdram.tile([2, F], f32, name="ccout")
                nc.sync.dma_start(ccin[0:1, :], sum_sb[:])
                nc.sync.dma_start(
                    ccin[1:2, :].rearrange("1 (i p) -> p i 1", p=128),
                    stat_sb[:, :, 1:2])
                if single_core_sim:
                    nc.gpsimd.dma_start(ccout[:], ccin[:])
                else:
                    nc.gpsimd.collective_compute(
                        "AllReduce", ALU.add,
                        replica_groups=[list(range(N_CORES))],
                        ins=[ccin.opt()], outs=[ccout.opt()])
                nc.sync.dma_start(
                    stat_sb[:, :, 0:1].rearrange("p i 1 -> p i"),
                    ccout[0:1, :].rearrange("1 (i p) -> p i", p=128))
                nc.sync.dma_start(
                    stat_sb[:, :, 1:2].rearrange("p i 1 -> p i"),
                    ccout[1:2, :].rearrange("1 (i p) -> p i", p=128))

                mom = pc.tile([128, NFT, 2], f32, name="mom")
                nc.vector.tensor_scalar(mom[:], stat_sb[:], 1.0 / B_TOTAL,
                                        None, op0=ALU.mult)
                mean = mom[:, :, 0]
                ex2 = mom[:, :, 1]
                var = pc.tile([128, NFT], f32, name="var")
                nc.vector.tensor_tensor(var[:], mean, mean, op=ALU.mult)
                nc.vector.tensor_tensor(var[:], ex2, var[:],
                                        op=ALU.subtract)
                eps = pc.tile([128, 1], f32, name="eps")
                nc.vector.memset(eps[:], BN_EPS)
                se = pc.tile([128, NFT], f32, name="se")
                nc.scalar.activation(se[:], var[:], AF.Sqrt, bias=eps[:])
                sinv = pc.tile([128, NFT], f32, name="sinv")
                nc.vector.reciprocal(sinv[:], se[:])
                av = pc.tile([128, NFT], f32, name="av")
                nc.vector.tensor_tensor(av[:], sinv[:], gamma2[:],
                                        op=ALU.mult)
                cv = pc.tile([128, NFT], f32, name="cv")
                nc.vector.tensor_tensor(cv[:], mean, av[:], op=ALU.mult)
                nc.vector.tensor_tensor(cv[:], beta2[:], cv[:],
                                        op=ALU.subtract)

                p2a = [pc.tile([128, TS], f16, tag=f"p2a{i}", name=f"p2a{i}")
                       for i in range(NFT)]
                for i in range(NFT):
                    nc.vector.tensor_scalar(p2a[i][:], p2raw[:, i, :],
                                            av[:, i:i + 1], None,
                                            op0=ALU.mult)
                biasA = pc.tile([128, NM], f32, name="biasA")
                with tc.tile_pool(name="dps", bufs=1, space="PSUM") as pdp:
                    dP = pdp.tile([128, NM], f32, name="dP")
                    for m in range(NM):
                        for i in range(NFT):
                            nc.tensor.matmul(
                                dP[:, m:m + 1],
                                p2raw[:, i, 128 * m:128 * (m + 1)],
                                cv[:, i:i + 1],
                                start=(i == 0), stop=(i == NFT - 1))
                    nc.vector.tensor_tensor(biasA[:], dP[:], cutv[:],
                                            op=ALU.subtract)

                # ---------- phase 2: software-pipelined tree forest ------
                with tc.tile_pool(name="z", bufs=3, space="PSUM") as pz, \
                     tc.tile_pool(name="outp", bufs=2, space="PSUM") as pop, \
                     tc.tile_pool(name="sc", bufs=3) as psc, \
                     tc.tile_pool(name="o1", bufs=3) as po1, \
                     tc.tile_pool(name="osb", bufs=3) as pos:
                    NJ = NCH * NM
                    scs, o1s, outPs = {}, {}, {}

                    def stageA(j):
                        c, m = divmod(j, NM)
                        zp = pz.tile([128, CHUNK], f32, tag="z", name="zp")
                        for i in range(NFT):
                            for q in range(Q):
                                nc.tensor.matmul(
                                    zp[:, 512 * q:512 * (q + 1)],
                                    p2a[i][:, 128 * m:128 * (m + 1)],
                                    xT[i][:, c * CHUNK + 512 * q:
                                          c * CHUNK + 512 * (q + 1)],
                                    start=(i == 0), stop=(i == NFT - 1),
                                    skip_group_check=True)
                        sc = psc.tile([128, CHUNK], f16, tag="sc", name="sc")
                        nc.scalar.activation(sc[:], zp[:], AF.Sigmoid,
                                             bias=biasA[:, m:m + 1])
                        scs[j] = sc

                    def stageB(j):
                        c, m = divmod(j, NM)
                        sc = scs.pop(j)
                        z2 = pz.tile([128, CHUNK], f32, tag="z", name="z2")
                        for q in range(Q):
                            nc.tensor.matmul(z2[:, 512 * q:512 * (q + 1)],
                                             w1bd[:, m, :],
                                             sc[:, 512 * q:512 * (q + 1)],
                                             start=True, stop=True)
                        o1 = po1.tile([128, CHUNK], f16, tag="o1", name="o1")
                        nc.vector.tensor_scalar(o1[:], z2[:], b1v[:, m:m + 1],
                                                0.0, op0=ALU.add, op1=ALU.max)
                        o1s[j] = o1

                    def stageC(j):
                        c, m = divmod(j, NM)
                        if m == 0:
                            outPs[c] = pop.tile([128, 512], f32, tag="outp",
                                                name=f"outp{c}")
                        o1 = o1s.pop(j)
                        for q in range(Q):
                            nc.tensor.matmul(
                                outPs[c][32 * q:32 * q + O, :], w2f[:, m, :],
                                o1[:, 512 * q:512 * (q + 1)],
                                start=(m == 0), stop=(m == NM - 1),
                                skip_group_check=True,
                                tile_position=(0, 32 * q))
                        if m == NM - 1:
                            for q in range(Q):
                                osb = pos.tile([O, 512], f32, tag="osb",
                                               name="osb")
                                src_ap = outPs[c][32 * q:32 * q + O, :]
                                if q % 2 == 0:
                                    nc.scalar.activation(osb[:], src_ap,
                                                         AF.Identity,
                                                         bias=bout[:])
                                else:
                                    nc.vector.tensor_scalar(
                                        osb[:], src_ap, bout[:],
                                        None, op0=ALU.add)
                                nc.sync.dma_start(
                                    OUT[:, c * CHUNK + 512 * q:
                                        c * CHUNK + 512 * (q + 1)], osb[:])
                            del outPs[c]

                    for j in range(NJ + 2):
                        if j < NJ:
                            stageA(j)
                        if 1 <= j < NJ + 1:
                            stageB(j - 1)
                        if j >= 2:
                            stageC(j - 2)

            for _rep in range(repeat):
                body_once()
    nc.compile()
    return nc


_NC_CACHE = {}


def _get_program(repeat=1):
    if repeat not in _NC_CACHE:
        _NC_CACHE[repeat] = build_program(repeat)
    return _NC_CACHE[repeat]


def make_in_maps(inputs):
    x = np.ascontiguousarray(inputs["x"], dtype=np.float32)
    params = _host_prep(np.asarray(inputs["gamma"]), np.asarray(inputs["beta"]),
                        np.asarray(inputs["fsm"]), np.asarray(inputs["cut"]),
                        np.asarray(inputs["W1"]), np.asarray(inputs["b1"]),
                        np.asarray(inputs["W2"]), np.asarray(inputs["b2"]),
                        np.asarray(inputs["tw"]))
    return [{"x": x[c * BS:(c + 1) * BS], **params} for c in range(N_CORES)]


def kernel(x, gamma, beta, fsm, cut, W1, b1, W2, b2, tw):
    """Full unsharded inputs in, full [B, O] float32 output out."""
    inputs = dict(x=x, gamma=gamma, beta=beta, fsm=fsm, cut=cut, W1=W1,
                  b1=b1, W2=W2, b2=b2, tw=tw)
    nc = _get_program(repeat=1)
    in_maps = make_in_maps(inputs)
    res = run_bass_kernel_spmd(nc, in_maps, core_ids=list(range(N_CORES)))
    out = np.concatenate([res.results[c]["out"] for c in range(N_CORES)],
                         axis=1)
    return np.ascontiguousarray(out.T, dtype=np.float32)



# revision 15
# speedup vs baseline: 1.5979x; 1.5979x over previous
"""nn_CART_69355131895963 Trainium2 Bass kernel.

reference:
    BatchNorm1d(train-mode batch stats) -> per-tree sparsemax feature
    selection (einsum bf,tfs->tbs) -> sigmoid(xp - cut) -> per-tree
    [S,S] MLP layer + relu -> per-tree [S,O] layer -> mean over trees of
    o2 * tw.

Strategy (8 NeuronCores, batch-sharded 8192 rows/core):
  Host (O(params) only): sparsemax(fsm) -> P2 [F,TS] in feature-PAIR
    layout [128,2,TS]; tw/T folded into W2; block-diagonal W1 (4
    trees/group); all small parameters laid out for direct SBUF use.
  Device phase 1 (streamed over 4 x 2048-row chunks):
    DMA x fp32 -> GPSIMD cast to fp8e4 -> BN stats on PE as fp8
    DoubleRow matmuls (cov diag + ones-sums, 2x K per instr) ->
    pair-transpose on PE (fp16 bitcast of fp8 pairs x identity) ->
    DVE evict psum->SBUF giving xT16 [128pairs, 8192] (= fp8 [f, b]).
  Phase 1.5: AllReduce [2,F] stats, finish mean/var -> a=gamma*rsqrt,
    fold a into P (p2a8 fp8 pair layout), bias dP-cut for the sigmoid.
  Phase 2 (8 chunks of 1024 cols, software-pipelined A/B stages):
    A: xp = DoubleRow-fp8 matmul (full F=256 contraction per instr)
       -> ACT sigmoid(xp + biasA) -> score fp16
    B: z2 = W1bd^T @ score (fp16 PE) -> relu+b1 on DVE/GPSIMD (split)
       -> o1 fp16 [128ts, 8m, 1024]
    C (per chunk): out^T[b,16] psum += o1(m)^T @ W2f(m) over 8 groups
       (16-col matmuls, 8.4x cheaper than [16,b] orientation) ->
       DVE evict + bout bias -> DMA rows to OUT [8192, 16].
  Host: concat per-core outputs along batch. No host-side transpose.
"""

import numpy as np

import concourse.tile as tile
from concourse import bacc, mybir
from concourse.bass_utils import run_bass_kernel_spmd

f8 = mybir.dt.float8e4
f16 = mybir.dt.float16
f32 = mybir.dt.float32
AF = mybir.ActivationFunctionType
ALU = mybir.AluOpType
DRM = mybir.MatmulPerfMode.DoubleRow

N_CORES = 8
B_TOTAL = 65536
BS = B_TOTAL // N_CORES     # 8192 rows per core
F = 256
T = 32
S = 32
O = 16
TS = T * S                  # 1024
NM = TS // 128              # 8 ts-tiles (tree groups of 4)
BN_EPS = 1e-5

ROWS1 = 2048                # phase-1 chunk rows
NCH1 = BS // ROWS1          # 4
SUB1 = ROWS1 // 128         # 16 b-subtiles per chunk

CH = 1024                   # phase-2 chunk columns
NCH = BS // CH              # 8


def _sparsemax_cols(z):
    """sparsemax along axis 0 of z [F, C] (float64)."""
    zs = np.sort(z, axis=0)[::-1]
    k = np.arange(1, z.shape[0] + 1)[:, None]
    cs = np.cumsum(zs, axis=0)
    support = (1.0 + k * zs) > cs
    ksup = support.sum(0)
    tau = (cs[ksup - 1, np.arange(z.shape[1])] - 1.0) / ksup
    return np.maximum(z - tau, 0.0)


def _host_prep(gamma, beta, fsm, cut, W1, b1, W2, b2, tw):
    import ml_dtypes
    P2 = _sparsemax_cols(
        fsm.astype(np.float64).transpose(1, 0, 2).reshape(F, TS)
    ).astype(np.float32)
    p2pr = np.ascontiguousarray(P2.reshape(128, 2, TS)).astype(np.float16)
    cutv = cut.reshape(TS).reshape(NM, 128).T.copy().astype(np.float32)
    b1v = b1.reshape(TS).reshape(NM, 128).T.copy().astype(np.float32)

    w1bd = np.zeros((NM, 128, 128), dtype=np.float32)
    for g in range(NM):
        for i in range(4):
            w1bd[g, 32 * i:32 * i + 32, 32 * i:32 * i + 32] = W1[4 * g + i]
    w1bd = w1bd.transpose(1, 0, 2).astype(np.float16).copy()

    w2f = (W2 * (tw / T)).reshape(TS, O).astype(np.float32) \
        .reshape(NM, 128, O).transpose(1, 0, 2).astype(np.float16).copy()
    bout = (b2 * (tw / T)).sum(0).reshape(O).astype(np.float32)
    boutbc = np.ascontiguousarray(
        np.broadcast_to(bout[None, None, :], (128, NM, O)), dtype=np.float32)

    gpair = gamma.reshape(128, 2).copy().astype(np.float32)
    bpair = beta.reshape(128, 2).copy().astype(np.float32)
    eye = np.eye(128, dtype=np.float32)
    eye16 = np.eye(128, dtype=np.float16)
    ones8 = np.ones((128, 2, 1), dtype=ml_dtypes.float8_e4m3)
    return dict(p2pr=p2pr, cutv=cutv, b1v=b1v, w1bd=w1bd, w2f=w2f,
                boutbc=boutbc, gpair=gpair, bpair=bpair, eye=eye,
                eye16=eye16, ones8=ones8)


def build_program(repeat=1, single_core_sim=False, LAG=2, CDELAY=2,
                  ACTRELU=13, SCBUFS=3, S3G=1, O1BUFS=2):
    """Trace + compile the SPMD Bass program (identical on all 8 cores).

    single_core_sim=True builds the same per-core program with the
    cross-core AllReduce elided (for cost-model simulation only).
    """
    ncores = 1 if single_core_sim else N_CORES
    nc = bacc.Bacc("TRN2", target_bir_lowering=False, debug=False,
                   num_devices=ncores)
    X = nc.dram_tensor("x", [BS, F], f32, kind="ExternalInput").ap()
    P2PR = nc.dram_tensor("p2pr", [128, 2, TS], f16, kind="ExternalInput").ap()
    CUTV = nc.dram_tensor("cutv", [128, NM], f32, kind="ExternalInput").ap()
    B1V = nc.dram_tensor("b1v", [128, NM], f32, kind="ExternalInput").ap()
    W1BD = nc.dram_tensor("w1bd", [128, NM, 128], f16, kind="ExternalInput").ap()
    W2F = nc.dram_tensor("w2f", [128, NM, O], f16, kind="ExternalInput").ap()
    BOUTBC = nc.dram_tensor("boutbc", [128, NM, O], f32, kind="ExternalInput").ap()
    GPAIR = nc.dram_tensor("gpair", [128, 2], f32, kind="ExternalInput").ap()
    BPAIR = nc.dram_tensor("bpair", [128, 2], f32, kind="ExternalInput").ap()
    EYE = nc.dram_tensor("eye", [128, 128], f32, kind="ExternalInput").ap()
    EYE16 = nc.dram_tensor("eye16", [128, 128], f16, kind="ExternalInput").ap()
    ONES8 = nc.dram_tensor("ones8", [128, 2, 1], f8, kind="ExternalInput").ap()
    OUT = nc.dram_tensor("out", [BS, O], f32, kind="ExternalOutput").ap()

    Xv = X.rearrange("(n p) f -> p n f", p=128)

    with tile.TileContext(nc) as tc:
        with tc.tile_pool(name="const", bufs=1) as pc, \
             tc.tile_pool(name="xt", bufs=1) as pxt, \
             tc.tile_pool(name="dram", bufs=1, space="DRAM") as pdram:

            def load_const(name, shape, dt, src):
                t = pc.tile(shape, dt, name=name)
                nc.sync.dma_start(t[:], src[:])
                return t

            # small consts needed early in phase 1
            eye = load_const("eye_sb", [128, 128], f32, EYE)
            eye16 = load_const("eye16_sb", [128, 128], f16, EYE16)
            ones8 = load_const("ones8_sb", [128, 2, 1], f8, ONES8)

            # big consts issued AFTER the x DMAs (loaded lazily below)
            big = {}

            def load_big_consts():
                big["p2pr"] = load_const("p2pr_sb", [128, 2, TS], f16, P2PR)
                big["cutv"] = load_const("cutv_sb", [128, NM], f32, CUTV)
                big["b1v"] = load_const("b1v_sb", [128, NM], f32, B1V)
                big["w1bd"] = load_const("w1bd_sb", [128, NM, 128], f16, W1BD)
                big["w2f"] = load_const("w2f_sb", [128, NM, O], f16, W2F)
                big["boutbc"] = load_const("boutbc_sb", [128, NM, O], f32,
                                           BOUTBC)
                big["gpair"] = load_const("gpair_sb", [128, 2], f32, GPAIR)
                big["bpair"] = load_const("bpair_sb", [128, 2], f32, BPAIR)

            # xT16[p, b] (fp16-typed) = fp8 pair (x[b, 2p], x[b, 2p+1])
            xT16 = pxt.tile([128, BS], f16, name="xt16")

            def body_once():
                # ---------- phase 1: load, cast fp8, stats, transpose ----
                stat_half = pc.tile([128, 2, 2], f32, name="stat_half")
                with tc.tile_pool(name="ph1", bufs=2) as p1, \
                     tc.tile_pool(name="x32p", bufs=NCH1) as p1x, \
                     tc.tile_pool(name="ph1ps", bufs=1, space="PSUM") as pst, \
                     tc.tile_pool(name="trps", bufs=2, space="PSUM") as ptr:
                    # issue the whole x load stream first: DMA is the
                    # serial prefix floor, nothing may queue ahead of it
                    x32s = []
                    for c in range(NCH1):
                        x32 = p1x.tile([128, SUB1, F], f32, tag="x32",
                                       name="x32")
                        nc.sync.dma_start(x32[:],
                                          Xv[:, c * SUB1:(c + 1) * SUB1, :])
                        x32s.append(x32)
                    if not big:
                        load_big_consts()
                    covP = [pst.tile([128, 128], f32, tag=f"cov{i}",
                                     name=f"cov{i}") for i in range(2)]
                    sumP = [pst.tile([128, 1], f32, tag=f"sum{i}",
                                     name=f"sum{i}") for i in range(2)]
                    for c in range(NCH1):
                        x32 = x32s[c]
                        x8 = p1.tile([128, SUB1, F], f8, tag="x8", name="x8")
                        # fp32 -> fp8e4 cast split across GPS/ACT/DVE
                        nc.gpsimd.tensor_copy(x8[:, 0:6, :], x32[:, 0:6, :])
                        nc.scalar.copy(x8[:, 6:12, :], x32[:, 6:12, :])
                        nc.vector.tensor_copy(x8[:, 12:16, :],
                                              x32[:, 12:16, :])
                        # stats: fp8 DoubleRow over b-subtile pairs
                        for a in range(SUB1 // 2):
                            first = (c == 0 and a == 0)
                            last = (c == NCH1 - 1 and a == SUB1 // 2 - 1)
                            sl = x8[:, 2 * a:2 * a + 2, :]
                            for i in range(2):
                                fs = sl[:, :, 128 * i:128 * (i + 1)]
                                nc.tensor.matmul(covP[i][:], fs, fs,
                                                 start=first, stop=last,
                                                 perf_mode=DRM,
                                                 skip_group_check=True)
                                nc.tensor.matmul(sumP[i][:], fs, ones8[:],
                                                 start=first, stop=last,
                                                 perf_mode=DRM,
                                                 skip_group_check=True)
                        # pair-transpose: 4 b-tiles per psum buf, then evict
                        for g in range(SUB1 // 4):
                            ztr = ptr.tile([128, 512], f16, tag="ztr",
                                           name="ztr")
                            for t in range(4):
                                bt = 4 * g + t
                                nc.tensor.matmul(
                                    ztr[:, 128 * t:128 * (t + 1)],
                                    x8[:, bt, :].bitcast(f16), eye16[:],
                                    is_transpose=True, start=True, stop=True,
                                    skip_group_check=True)
                            col = c * ROWS1 + g * 512
                            if g % 2 == 0:
                                nc.vector.tensor_copy(xT16[:, col:col + 512],
                                                      ztr[:])
                            else:
                                nc.scalar.copy(xT16[:, col:col + 512],
                                               ztr[:])
                    # gather stats: [128, 2(i), 2(kind)] in F-half layout
                    for i in range(2):
                        tmp = p1.tile([128, 128], f32, tag="dtmp", name="dtmp")
                        nc.vector.tensor_tensor(tmp[:], covP[i][:], eye[:],
                                                op=ALU.mult)
                        nc.vector.reduce_sum(stat_half[:, i, 1:2], tmp[:],
                                             axis=mybir.AxisListType.X)
                        nc.vector.tensor_copy(stat_half[:, i, 0:1],
                                              sumP[i][:])

                # ---------- phase 1.5: all-reduce + BN fold ----------
                # CC buffer is f-major [F, 2] so each leg is ONE dma call
                ccin = pdram.tile([F, 2], f32, name="ccin")
                ccout = pdram.tile([F, 2], f32, name="ccout")
                # f = 128*i + p (F-half layout) on the way out
                nc.sync.dma_start(
                    ccin[:].rearrange("(i p) k -> p i k", p=128),
                    stat_half[:])
                if single_core_sim:
                    ccred = ccin   # collective elided: read partials back
                else:
                    nc.gpsimd.collective_compute(
                        "AllReduce", ALU.add,
                        replica_groups=[list(range(N_CORES))],
                        ins=[ccin.opt()], outs=[ccout.opt()])
                    ccred = ccout
                # read back in PAIR layout: f = 2p + j
                stat_pair = pc.tile([128, 2, 2], f32, name="stat_pair")
                nc.sync.dma_start(
                    stat_pair[:],
                    ccred[:].rearrange("(p j) k -> p j k", p=128))

                p2pr, cutv, b1v = big["p2pr"], big["cutv"], big["b1v"]
                w1bd, w2f, boutbc = big["w1bd"], big["w2f"], big["boutbc"]

                mom = pc.tile([128, 2, 2], f32, name="mom")
                nc.vector.tensor_scalar(mom[:], stat_pair[:], 1.0 / B_TOTAL,
                                        None, op0=ALU.mult)
                mean = mom[:, :, 0]
                ex2 = mom[:, :, 1]
                var = pc.tile([128, 2], f32, name="var")
                nc.vector.tensor_tensor(var[:], mean, mean, op=ALU.mult)
                nc.vector.tensor_tensor(var[:], ex2, var[:], op=ALU.subtract)
                eps = pc.tile([128, 1], f32, name="eps")
                nc.vector.memset(eps[:], BN_EPS)
                se = pc.tile([128, 2], f32, name="se")
                nc.scalar.activation(se[:], var[:], AF.Sqrt, bias=eps[:])
                # dummy sigmoid: trigger the act-table switch now so the
                # 1.28us load overlaps the rest of the fold
                dumm = pc.tile([128, 1], f16, name="dumm")
                nc.scalar.activation(dumm[:], se[:, 0:1], AF.Sigmoid)
                sinv = pc.tile([128, 2], f32, name="sinv")
                nc.vector.reciprocal(sinv[:], se[:])
                av = pc.tile([128, 2], f32, name="av")
                nc.vector.tensor_tensor(av[:], sinv[:], big["gpair"][:],
                                        op=ALU.mult)
                cv = pc.tile([128, 2], f32, name="cv")
                nc.vector.tensor_tensor(cv[:], mean, av[:], op=ALU.mult)
                nc.vector.tensor_tensor(cv[:], big["bpair"][:], cv[:],
                                        op=ALU.subtract)
                cv16 = pc.tile([128, 2], f16, name="cv16")
                nc.vector.tensor_copy(cv16[:], cv[:])

                p2a8 = pc.tile([128, 2, TS], f8, name="p2a8")
                nc.vector.tensor_scalar(p2a8[:, 0, :], p2pr[:, 0, :],
                                        av[:, 0:1], None, op0=ALU.mult)
                nc.scalar.activation(p2a8[:, 1, :], p2pr[:, 1, :], AF.Copy,
                                     scale=av[:, 1:2])
                biasA = pc.tile([128, NM], f32, name="biasA")
                with tc.tile_pool(name="dps", bufs=1, space="PSUM") as pdp:
                    dP = pdp.tile([128, NM], f32, name="dP")
                    for m in range(NM):
                        for j in range(2):
                            nc.tensor.matmul(
                                dP[:, m:m + 1],
                                p2pr[:, j, 128 * m:128 * (m + 1)],
                                cv16[:, j:j + 1],
                                start=(j == 0), stop=(j == 1),
                                skip_group_check=True)
                    nc.vector.tensor_tensor(biasA[:], dP[:], cutv[:],
                                            op=ALU.subtract)

                # fp8 view of xT16: [p, j, b] with j the feature-pair lane
                xT8 = xT16[:].bitcast(f8).rearrange("p (b j) -> p j b", j=2)

                # ---------- phase 2: software-pipelined tree forest ------
                with tc.tile_pool(name="xpps", bufs=2, space="PSUM") as pxp, \
                     tc.tile_pool(name="z2ps", bufs=2, space="PSUM") as pz2, \
                     tc.tile_pool(name="sc", bufs=SCBUFS) as psc, \
                     tc.tile_pool(name="o1", bufs=O1BUFS) as po1, \
                     tc.tile_pool(name="osb", bufs=2) as pos:
                    NJ = NCH * NM
                    scs, o1cs = {}, {}

                    def stageA(j):
                        c, m = divmod(j, NM)
                        xp = pxp.tile([128, CH], f32, tag="xp", name="xp")
                        for q in range(CH // 512):
                            nc.tensor.matmul(
                                xp[:, 512 * q:512 * (q + 1)],
                                p2a8[:, :, 128 * m:128 * (m + 1)],
                                xT8[:, :, c * CH + 512 * q:
                                    c * CH + 512 * (q + 1)],
                                start=True, stop=True, perf_mode=DRM,
                                skip_group_check=True)
                        sc = psc.tile([128, CH], f16, tag="sc", name="sc")
                        nc.scalar.activation(sc[:], xp[:], AF.Sigmoid,
                                             bias=biasA[:, m:m + 1])
                        scs[j] = sc

                    def stageB(j):
                        c, m = divmod(j, NM)
                        sc = scs.pop(j)
                        if m == 0:
                            o1cs[c] = po1.tile([128, NM, CH], f16, tag="o1",
                                               name=f"o1c{c}")
                        z2 = pz2.tile([128, CH], f32, tag="z2", name="z2")
                        for q in range(CH // 512):
                            nc.tensor.matmul(z2[:, 512 * q:512 * (q + 1)],
                                             w1bd[:, m, :],
                                             sc[:, 512 * q:512 * (q + 1)],
                                             start=True, stop=True,
                                             skip_group_check=True)
                        # GPSIMD cannot read PSUM; split relu DVE/ACT
                        if ACTRELU and j % ACTRELU == ACTRELU - 1:
                            nc.scalar.activation(o1cs[c][:, m, :], z2[:],
                                                 AF.Relu,
                                                 bias=b1v[:, m:m + 1])
                        else:
                            nc.vector.tensor_scalar(o1cs[c][:, m, :], z2[:],
                                                    b1v[:, m:m + 1], 0.0,
                                                    op0=ALU.add, op1=ALU.max)

                    def stageC(cg):
                        # one psum borrow (z2-tag) covers S3G chunks
                        outT = pz2.tile([128, S3G, NM, O], f32, tag="z2",
                                        name=f"outT{cg}")
                        for ci in range(S3G):
                            c = cg * S3G + ci
                            o1c = o1cs.pop(c)
                            for bt in range(CH // 128):
                                for m in range(NM):
                                    nc.tensor.matmul(
                                        outT[:, ci, bt, :],
                                        o1c[:, m, 128 * bt:128 * (bt + 1)],
                                        w2f[:, m, :],
                                        start=(m == 0), stop=(m == NM - 1),
                                        skip_group_check=True)
                        osb = pos.tile([128, S3G, NM, O], f32, tag="osb",
                                       name="osb")
                        for ci in range(S3G):
                            nc.vector.tensor_tensor(osb[:, ci, :, :],
                                                    outT[:, ci, :, :],
                                                    boutbc[:], op=ALU.add)
                        cg0 = cg * S3G * CH
                        nc.sync.dma_start(
                            OUT[cg0:cg0 + S3G * CH, :]
                               .rearrange("(s p) o -> p s o", p=128),
                            osb[:].rearrange("p g s o -> p (g s) o"))

                    # lag stageB so its PE matmuls never park in the
                    # 4-deep wait queue and block s1 issue (ACT starvation)
                    for j in range(NJ + LAG + CDELAY):
                        if j < NJ:
                            stageA(j)
                        jb = j - LAG
                        if 0 <= jb < NJ:
                            stageB(jb)
                        jc = j - LAG - CDELAY
                        if jc >= 0 and jc % (NM * S3G) == NM * S3G - 1:
                            stageC(jc // (NM * S3G))

            for _rep in range(repeat):
                body_once()
    nc.compile()
    return nc


_NC_CACHE = {}


def _get_program(repeat=1):
    if repeat not in _NC_CACHE:
        _NC_CACHE[repeat] = build_program(repeat)
    return _NC_CACHE[repeat]


def make_in_maps(inputs):
    x = np.ascontiguousarray(inputs["x"], dtype=np.float32)
    params = _host_prep(np.asarray(inputs["gamma"]), np.asarray(inputs["beta"]),
                        np.asarray(inputs["fsm"]), np.asarray(inputs["cut"]),
                        np.asarray(inputs["W1"]), np.asarray(inputs["b1"]),
                        np.asarray(inputs["W2"]), np.asarray(inputs["b2"]),
                        np.asarray(inputs["tw"]))
    return [{"x": x[c * BS:(c + 1) * BS], **params} for c in range(N_CORES)]


def kernel(x, gamma, beta, fsm, cut, W1, b1, W2, b2, tw):
    """Full unsharded inputs in, full [B, O] float32 output out."""
    inputs = dict(x=x, gamma=gamma, beta=beta, fsm=fsm, cut=cut, W1=W1,
                  b1=b1, W2=W2, b2=b2, tw=tw)
    nc = _get_program(repeat=1)
    in_maps = make_in_maps(inputs)
    res = run_bass_kernel_spmd(nc, in_maps, core_ids=list(range(N_CORES)))
    out = np.concatenate([res.results[c]["out"] for c in range(N_CORES)],
                         axis=0)
    return np.ascontiguousarray(out, dtype=np.float32)
